# revision 1
# baseline (speedup 1.0000x reference)
"""BilateralFilter (SqueezeSeg mc condensing-kernel gaussians) on 8 TRN2 cores.

Reference computes, for x: [16, 64, 512, 3] (B, Z, A, C=xyz):
    nbr   = 14 spatial neighbors of each pixel in a 3x5 window (zero-padded)
    diff2 = sum_c (x - nbr)^2                           [B, Z, A, 14]
    out   = exp(-diff2 / (2 * theta_r^2))               [B, Z, A, 14, 4]
with THETA_R = [0.015, 0.015, 0.01, 0.01] (only 2 distinct values).

Strategy (pure batch data-parallel, 2 batches per core):
  - partitions p = b*64 + z  (128), free dim = azimuth chunks of 256 (x2).
  - mirror symmetry: m_k(q) = |x(q) - x(q+off_k)|^2 for the 7 "negative"
    offsets k=0..6 gives the other 7 via diff2_{13-k}(q) = m_k(q - off_k);
    the z+1-partition read is materialized by an SBUF->SBUF partition-remap
    DMA (M_up), with the phantom z=64 row filled from s = |x|^2 by a
    stride-0 DMA (out-of-image neighbor => diff2 = |x(center)|^2).
  - ACT computes exp with the free scale immediate; each exp is written to
    both classes of its theta pair via a stride-0 input axis.
  - output staged in SBUF exactly in DRAM layout [a, k, c] so the store DMA
    is fully contiguous per partition (57 KB/partition runs).
"""

import numpy as np

import concourse.bass as bass
import concourse.tile as tile
from concourse import bacc, mybir
from concourse.bass_utils import run_bass_kernel_spmd

N_CORES = 8
B, Z, A, C = 16, 64, 512, 3
K, NCLS = 14, 4
LB = B // N_CORES            # local batches per core = 2
P = LB * Z                   # 128 partitions
AC = 256                     # azimuth chunk
NCH = A // AC                # 2 chunks
XW = AC + 8                  # x window (halo 4 each side)
MW = AC + 4                  # m window (halo 2 each side)
F32 = mybir.dt.float32

# exp scales: -1 / (2 * theta^2), theta pairs (0.015, 0.01), f32 semantics
_t0 = np.float32(0.015)
_t1 = np.float32(0.01)
SC0 = -float(1.0 / np.float32(np.float32(2.0) * _t0 * _t0))
SC1 = -float(1.0 / np.float32(np.float32(2.0) * _t1 * _t1))

# DRAM strides (elements) of out [LB, Z, A, K, NCLS]
O_A = K * NCLS               # 56
O_Z = A * O_A                # 28672
O_B = Z * O_Z                # 1835008
X_Z = A * C                  # 1536
X_B = Z * X_Z


def _ap(t, poff, pcnt, foff, pairs, pstep=1):
    """AP on tile t: partitions [poff, poff+pcnt) (stride pstep rows), free
    `pairs` ([step, count] in elements) based at element foff."""
    row = t.ap[0][0]
    return bass.AP(tensor=t.tensor, offset=t.offset + poff * row + foff,
                   ap=[[pstep * row, pcnt]] + [list(p) for p in pairs])


def _build():
    nc = bacc.Bacc("TRN2", target_bir_lowering=False, debug=False,
                   num_devices=N_CORES)
    x_h = nc.dram_tensor("x", [LB, Z, A, C], F32, kind="ExternalInput")
    o_h = nc.dram_tensor("out", [LB, Z, A, K, NCLS], F32, kind="ExternalOutput")
    x_ap, o_ap = x_h.ap(), o_h.ap()

    with tile.TileContext(nc) as tc:
        with tc.tile_pool(name="pool", bufs=2) as pool:
            for ci in range(NCH):
                a0 = ci * AC
                lo, hi = max(0, a0 - 4), min(A, a0 + AC + 4)
                c_lo = (lo - (a0 - 4)) * C          # first valid xt col
                c_hi = (hi - (a0 - 4)) * C

                # ---- load x window (zero halo at image borders) ----
                xt = pool.tile([P, XW * C], F32, name="xt")
                if c_lo > 0:
                    nc.vector.memset(_ap(xt, 0, P, 0, [[1, c_lo]]), 0.0)
                if c_hi < XW * C:
                    nc.vector.memset(
                        _ap(xt, 0, P, c_hi, [[1, XW * C - c_hi]]), 0.0)
                for b in range(LB):
                    nc.sync.dma_start(
                        _ap(xt, b * Z, Z, c_lo, [[C, hi - lo], [1, C]]),
                        bass.AP(tensor=x_ap.tensor, offset=b * X_B + lo * C,
                                ap=[[X_Z, Z], [C, hi - lo], [1, C]]))

                # ---- x_dn[p] = xt[p-1] (z-1 row), zeros at z=0 ----
                x_dn = pool.tile([P, XW * C], F32, name="x_dn")
                nc.vector.memset(x_dn[:], 0.0)
                for b in range(LB):
                    nc.sync.dma_start(
                        _ap(x_dn, b * Z + 1, Z - 1, 0, [[1, XW * C]]),
                        _ap(xt, b * Z, Z - 1, 0, [[1, XW * C]]))

                # ---- s = sum_c x^2 over the full x window ----
                sqx = pool.tile([P, XW * C], F32, name="sqx")
                nc.scalar.square(sqx[:], xt[:])
                st = pool.tile([P, XW], F32, name="st")
                nc.vector.tensor_reduce(
                    st[:], _ap(sqx, 0, P, 0, [[C, XW], [1, C]]),
                    axis=mybir.AxisListType.X, op=mybir.AluOpType.add)

                # ---- m_k maps: M[p, k*MW + ar] over a-window [a0-2, a0+258)
                # k=0..4: dz=-1, da=k-2 ; k=5,6: dz=0, da=k-7
                M = pool.tile([P, 7 * MW], F32, name="M")
                dt5 = pool.tile([P, 5 * MW * C], F32, name="dt5")
                nc.vector.tensor_tensor(
                    _ap(dt5, 0, P, 0, [[MW * C, 5], [C, MW], [1, C]]),
                    _ap(xt, 0, P, 2 * C, [[0, 5], [C, MW], [1, C]]),
                    _ap(x_dn, 0, P, 0, [[C, 5], [C, MW], [1, C]]),
                    op=mybir.AluOpType.subtract)
                nc.vector.tensor_tensor(
                    _ap(dt5, 0, P, 0, [[1, 5 * MW * C]]),
                    _ap(dt5, 0, P, 0, [[1, 5 * MW * C]]),
                    _ap(dt5, 0, P, 0, [[1, 5 * MW * C]]),
                    op=mybir.AluOpType.mult)
                nc.vector.tensor_reduce(
                    _ap(M, 0, P, 0, [[1, 5 * MW]]),
                    _ap(dt5, 0, P, 0, [[C, 5 * MW], [1, C]]),
                    axis=mybir.AxisListType.X, op=mybir.AluOpType.add)

                dt2 = pool.tile([P, 2 * MW * C], F32, name="dt2")
                nc.vector.tensor_tensor(
                    _ap(dt2, 0, P, 0, [[MW * C, 2], [C, MW], [1, C]]),
                    _ap(xt, 0, P, 2 * C, [[0, 2], [C, MW], [1, C]]),
                    _ap(xt, 0, P, 0, [[C, 2], [C, MW], [1, C]]),
                    op=mybir.AluOpType.subtract)
                nc.scalar.square(dt2[:], dt2[:])
                nc.vector.tensor_reduce(
                    _ap(M, 0, P, 5 * MW, [[1, 2 * MW]]),
                    _ap(dt2, 0, P, 0, [[C, 2 * MW], [1, C]]),
                    axis=mybir.AxisListType.X, op=mybir.AluOpType.add)

                # ---- M_up[p] = M[p+1] for k=0..4 cols; phantom z=64 rows
                # ({63,127}) = s(z=63 row) with k-dependent a-shift ----
                M_up = pool.tile([P, 5 * MW], F32, name="M_up")
                for b in range(LB):
                    nc.sync.dma_start(
                        _ap(M_up, b * Z, Z - 1, 0, [[1, 5 * MW]]),
                        _ap(M, b * Z + 1, Z - 1, 0, [[1, 5 * MW]]))
                # phantom: M_up[{63,127}, k*MW + ar] = st[{63,127}, ar + k]
                nc.sync.dma_start(
                    _ap(M_up, Z - 1, 2, 0, [[MW, 5], [1, MW]], pstep=Z),
                    _ap(st, Z - 1, 2, 0, [[1, 5], [1, MW]], pstep=Z))

                # ---- exps into O staging [p, ar*56 + k*4 + c] ----
                O = pool.tile([P, AC * O_A], F32, name="O")
                for th, sc in ((0, SC0), (1, SC1)):
                    co = 2 * th
                    # direct k=0..6: in M[p, k*MW + ar + 2]
                    nc.scalar.activation(
                        _ap(O, 0, P, co, [[4, 7], [O_A, AC], [1, 2]]),
                        _ap(M, 0, P, 2, [[MW, 7], [1, AC], [0, 2]]),
                        mybir.ActivationFunctionType.Exp, scale=sc)
                    # a-mirrors k'=7,8 <- k=6,5: col = k*MW + ar + (9-k)
                    nc.scalar.activation(
                        _ap(O, 0, P, 28 + co, [[4, 2], [O_A, AC], [1, 2]]),
                        _ap(M, 0, P, 6 * MW + 3, [[-(MW - 1), 2], [1, AC], [0, 2]]),
                        mybir.ActivationFunctionType.Exp, scale=sc)
                    # dz-mirrors k'=9..13 <- k=4..0: M_up[p, k*MW + ar + 4 - k]
                    nc.scalar.activation(
                        _ap(O, 0, P, 36 + co, [[4, 5], [O_A, AC], [1, 2]]),
                        _ap(M_up, 0, P, 4 * (MW - 1) + 4,
                            [[-(MW - 1), 5], [1, AC], [0, 2]]),
                        mybir.ActivationFunctionType.Exp, scale=sc)

                # ---- store ----
                for b in range(LB):
                    nc.sync.dma_start(
                        bass.AP(tensor=o_ap.tensor,
                                offset=b * O_B + a0 * O_A,
                                ap=[[O_Z, Z], [1, AC * O_A]]),
                        _ap(O, b * Z, Z, 0, [[1, AC * O_A]]))

    nc.compile()
    return nc


_NC = None


def _get_nc():
    global _NC
    if _NC is None:
        _NC = _build()
    return _NC


def kernel(x: np.ndarray) -> np.ndarray:
    x = np.ascontiguousarray(np.asarray(x, dtype=np.float32))
    assert x.shape == (B, Z, A, C), x.shape
    nc = _get_nc()
    in_maps = [{"x": x[i * LB:(i + 1) * LB]} for i in range(N_CORES)]
    res = run_bass_kernel_spmd(nc, in_maps, list(range(N_CORES)))
    return np.concatenate([res.results[i]["out"] for i in range(N_CORES)],
                          axis=0)


# revision 3
# speedup vs baseline: 64.0262x; 64.0262x over previous
"""BilateralFilter (SqueezeSeg mc condensing-kernel gaussians) on 8 TRN2 cores.

Reference computes, for x: [16, 64, 512, 3] (B, Z, A, C=xyz):
    nbr   = 14 spatial neighbors of each pixel in a 3x5 window (zero-padded)
    diff2 = sum_c (x - nbr)^2                           [B, Z, A, 14]
    out   = exp(-diff2 / (2 * theta_r^2))               [B, Z, A, 14, 4]
with THETA_R = [0.015, 0.015, 0.01, 0.01] (only 2 distinct values).

Strategy (pure batch data-parallel, 2 batches per core):
  - partitions p = b*64 + z  (128), free dim = azimuth chunks of 256 (x2).
  - mirror symmetry: m_k(q) = |x(q) - x(q+off_k)|^2 for the 7 "negative"
    offsets k=0..6 gives the other 7 via diff2_{13-k}(q) = m_k(q - off_k);
    the z+1-partition read is materialized by an SBUF->SBUF partition-remap
    DMA (M_up), with the phantom z=64 row filled from s = |x|^2 by a
    stride-0 DMA (out-of-image neighbor => diff2 = |x(center)|^2).
  - ACT computes exp with the free scale immediate; each exp is written to
    both classes of its theta pair via a stride-0 input axis.
  - output staged in SBUF exactly in DRAM layout [a, k, c] so the store DMA
    is fully contiguous per partition (57 KB/partition runs).
"""

import numpy as np

import concourse.bass as bass
import concourse.tile as tile
from concourse import bacc, mybir
from concourse.bass_utils import run_bass_kernel_spmd

N_CORES = 8
B, Z, A, C = 16, 64, 512, 3
K, NCLS = 14, 4
LB = B // N_CORES            # local batches per core = 2
P = LB * Z                   # 128 partitions
AC = 256                     # azimuth chunk
NCH = A // AC                # 2 chunks
XW = AC + 8                  # x window (halo 4 each side)
MW = AC + 4                  # m window (halo 2 each side)
F32 = mybir.dt.float32

# exp scales: -1 / (2 * theta^2), theta pairs (0.015, 0.01), f32 semantics
_t0 = np.float32(0.015)
_t1 = np.float32(0.01)
SC0 = -float(1.0 / np.float32(np.float32(2.0) * _t0 * _t0))
SC1 = -float(1.0 / np.float32(np.float32(2.0) * _t1 * _t1))

# DRAM strides (elements) of out [LB, Z, A, K, NCLS]
O_A = K * NCLS               # 56
O_Z = A * O_A                # 28672
O_B = Z * O_Z                # 1835008
X_Z = A * C                  # 1536
X_B = Z * X_Z


def _ap(t, poff, pcnt, foff, pairs, pstep=1):
    """AP on tile t: partitions [poff, poff+pcnt) (stride pstep rows), free
    `pairs` ([step, count] in elements) based at element foff."""
    row = t.ap[0][0]
    return bass.AP(tensor=t.tensor, offset=t.offset + poff * row + foff,
                   ap=[[pstep * row, pcnt]] + [list(p) for p in pairs])


def _build():
    nc = bacc.Bacc("TRN2", target_bir_lowering=False, debug=False,
                   num_devices=N_CORES)
    x_h = nc.dram_tensor("x", [LB, Z, A, C], F32, kind="ExternalInput")
    o_h = nc.dram_tensor("out", [LB, Z, A, K, NCLS], F32, kind="ExternalOutput")
    x_ap, o_ap = x_h.ap(), o_h.ap()

    with tile.TileContext(nc) as tc:
        with tc.tile_pool(name="pool", bufs=2) as pool:
            for ci in range(NCH):
                a0 = ci * AC
                lo, hi = max(0, a0 - 4), min(A, a0 + AC + 4)
                c_lo = (lo - (a0 - 4)) * C          # first valid xt col
                c_hi = (hi - (a0 - 4)) * C

                # ---- load x window (zero halo at image borders) ----
                xt = pool.tile([P, XW * C], F32, name="xt")
                if c_lo > 0:
                    nc.vector.memset(_ap(xt, 0, P, 0, [[1, c_lo]]), 0.0)
                if c_hi < XW * C:
                    nc.vector.memset(
                        _ap(xt, 0, P, c_hi, [[1, XW * C - c_hi]]), 0.0)
                for b in range(LB):
                    nc.sync.dma_start(
                        _ap(xt, b * Z, Z, c_lo, [[C, hi - lo], [1, C]]),
                        bass.AP(tensor=x_ap.tensor, offset=b * X_B + lo * C,
                                ap=[[X_Z, Z], [C, hi - lo], [1, C]]))

                # ---- x_dn[p] = xt[p-1] (z-1 row), zeros at z=0 ----
                x_dn = pool.tile([P, XW * C], F32, name="x_dn")
                nc.vector.memset(x_dn[:], 0.0)
                for b in range(LB):
                    nc.sync.dma_start(
                        _ap(x_dn, b * Z + 1, Z - 1, 0, [[1, XW * C]]),
                        _ap(xt, b * Z, Z - 1, 0, [[1, XW * C]]))

                # ---- s = sum_c x^2 over the full x window ----
                sqx = pool.tile([P, XW * C], F32, name="sqx")
                nc.scalar.square(sqx[:], xt[:])
                st = pool.tile([P, XW], F32, name="st")
                nc.vector.tensor_reduce(
                    st[:], _ap(sqx, 0, P, 0, [[C, XW], [1, C]]),
                    axis=mybir.AxisListType.X, op=mybir.AluOpType.add)

                # ---- m_k maps: M[p, k*MW + ar] over a-window [a0-2, a0+258)
                # k=0..4: dz=-1, da=k-2 ; k=5,6: dz=0, da=k-7
                M = pool.tile([P, 7 * MW], F32, name="M")
                dt5 = pool.tile([P, 5 * MW * C], F32, name="dt5")
                nc.vector.tensor_tensor(
                    _ap(dt5, 0, P, 0, [[MW * C, 5], [C, MW], [1, C]]),
                    _ap(xt, 0, P, 2 * C, [[0, 5], [C, MW], [1, C]]),
                    _ap(x_dn, 0, P, 0, [[C, 5], [C, MW], [1, C]]),
                    op=mybir.AluOpType.subtract)
                nc.vector.tensor_tensor(
                    _ap(dt5, 0, P, 0, [[1, 5 * MW * C]]),
                    _ap(dt5, 0, P, 0, [[1, 5 * MW * C]]),
                    _ap(dt5, 0, P, 0, [[1, 5 * MW * C]]),
                    op=mybir.AluOpType.mult)
                nc.vector.tensor_reduce(
                    _ap(M, 0, P, 0, [[1, 5 * MW]]),
                    _ap(dt5, 0, P, 0, [[C, 5 * MW], [1, C]]),
                    axis=mybir.AxisListType.X, op=mybir.AluOpType.add)

                dt2 = pool.tile([P, 2 * MW * C], F32, name="dt2")
                nc.vector.tensor_tensor(
                    _ap(dt2, 0, P, 0, [[MW * C, 2], [C, MW], [1, C]]),
                    _ap(xt, 0, P, 2 * C, [[0, 2], [C, MW], [1, C]]),
                    _ap(xt, 0, P, 0, [[C, 2], [C, MW], [1, C]]),
                    op=mybir.AluOpType.subtract)
                nc.scalar.square(dt2[:], dt2[:])
                nc.vector.tensor_reduce(
                    _ap(M, 0, P, 5 * MW, [[1, 2 * MW]]),
                    _ap(dt2, 0, P, 0, [[C, 2 * MW], [1, C]]),
                    axis=mybir.AxisListType.X, op=mybir.AluOpType.add)

                # ---- M_up[p] = M[p+1] for k=0..4 cols; phantom z=64 rows
                # ({63,127}) = s(z=63 row) with k-dependent a-shift ----
                M_up = pool.tile([P, 5 * MW], F32, name="M_up")
                for b in range(LB):
                    nc.sync.dma_start(
                        _ap(M_up, b * Z, Z - 1, 0, [[1, 5 * MW]]),
                        _ap(M, b * Z + 1, Z - 1, 0, [[1, 5 * MW]]))
                # phantom: M_up[{63,127}, k*MW + ar] = st[{63,127}, ar + k]
                nc.sync.dma_start(
                    _ap(M_up, Z - 1, 2, 0, [[MW, 5], [1, MW]], pstep=Z),
                    _ap(st, Z - 1, 2, 0, [[1, 5], [1, MW]], pstep=Z))

                # ---- exps into O staging [p, ar*56 + k*4 + c] ----
                O = pool.tile([P, AC * O_A], F32, name="O")
                for th, sc in ((0, SC0), (1, SC1)):
                    co = 2 * th
                    # direct k=0..6: in M[p, k*MW + ar + 2]
                    nc.scalar.activation(
                        _ap(O, 0, P, co, [[4, 7], [O_A, AC], [1, 2]]),
                        _ap(M, 0, P, 2, [[MW, 7], [1, AC], [0, 2]]),
                        mybir.ActivationFunctionType.Exp, scale=sc)
                    # a-mirrors k'=7,8 <- k=6,5: col = k*MW + ar + (9-k)
                    nc.scalar.activation(
                        _ap(O, 0, P, 28 + co, [[4, 2], [O_A, AC], [1, 2]]),
                        _ap(M, 0, P, 6 * MW + 3, [[-(MW - 1), 2], [1, AC], [0, 2]]),
                        mybir.ActivationFunctionType.Exp, scale=sc)
                    # dz-mirrors k'=9..13 <- k=4..0: M_up[p, k*MW + ar + 4 - k]
                    nc.scalar.activation(
                        _ap(O, 0, P, 36 + co, [[4, 5], [O_A, AC], [1, 2]]),
                        _ap(M_up, 0, P, 4 * (MW - 1) + 4,
                            [[-(MW - 1), 5], [1, AC], [0, 2]]),
                        mybir.ActivationFunctionType.Exp, scale=sc)

                # ---- store ----
                for b in range(LB):
                    nc.sync.dma_start(
                        bass.AP(tensor=o_ap.tensor,
                                offset=b * O_B + a0 * O_A,
                                ap=[[O_Z, Z], [1, AC * O_A]]),
                        _ap(O, b * Z, Z, 0, [[1, AC * O_A]]))

    nc.compile()
    return nc


class _Runner:
    """Compile once; reuse the jitted sharded executable across calls.

    Mirrors bass2jax.run_bass_via_pjrt's multi-core path, but without
    donated output buffers (the kernel writes every output element, so the
    zero "output operands" are passed once from device-resident buffers and
    reused)."""

    def __init__(self):
        import jax
        from jax.sharding import Mesh, PartitionSpec, NamedSharding
        try:
            from jax.experimental.shard_map import shard_map
        except ImportError:
            from jax.shard_map import shard_map  # newer jax
        from concourse import bass2jax

        bass2jax.install_neuronx_cc_hook()
        nc = _build()
        self.nc = nc

        partition_name = (nc.partition_id_tensor.name
                          if nc.partition_id_tensor else None)
        in_names, out_names, out_avals = [], [], []
        for alloc in nc.m.functions[0].allocations:
            if not isinstance(alloc, mybir.MemoryLocationSet):
                continue
            name = alloc.memorylocations[0].name
            if alloc.kind == "ExternalInput":
                if name != partition_name:
                    in_names.append(name)
            elif alloc.kind == "ExternalOutput":
                out_names.append(name)
                out_avals.append(jax.core.ShapedArray(
                    tuple(alloc.tensor_shape), mybir.dt.np(alloc.dtype)))
        assert in_names == ["x"] and out_names == ["out"], (in_names, out_names)
        all_in_names = in_names + out_names
        if partition_name is not None:
            all_in_names = all_in_names + [partition_name]

        def _body(*args):
            operands = list(args)
            if partition_name is not None:
                operands.append(bass2jax.partition_id_tensor())
            return tuple(bass2jax._bass_exec_p.bind(
                *operands,
                out_avals=tuple(out_avals),
                in_names=tuple(all_in_names),
                out_names=tuple(out_names),
                lowering_input_output_aliases=(),
                sim_require_finite=True,
                sim_require_nnan=True,
                nc=nc,
            ))

        devices = jax.devices()[:N_CORES]
        assert len(devices) == N_CORES
        self.mesh = Mesh(np.asarray(devices), ("core",))
        spec = PartitionSpec("core")
        self.sharding = NamedSharding(self.mesh, spec)
        self.jitted = jax.jit(shard_map(
            _body, mesh=self.mesh, in_specs=(spec, spec), out_specs=(spec,),
            check_rep=False))
        # device-resident dummy output operand, created once
        self.zeros_dev = jax.device_put(
            np.zeros((N_CORES * LB, Z, A, K, NCLS), np.float32), self.sharding)
        self._jax = jax

    def put(self, x: np.ndarray):
        return self._jax.device_put(
            np.ascontiguousarray(np.asarray(x, np.float32)), self.sharding)

    def run_dev(self, x_dev):
        """Execute; returns device array (not fetched)."""
        return self.jitted(x_dev, self.zeros_dev)[0]

    def __call__(self, x: np.ndarray) -> np.ndarray:
        return np.asarray(self.run_dev(self.put(x)))


_RUNNER = None


def _get_runner():
    global _RUNNER
    if _RUNNER is None:
        _RUNNER = _Runner()
    return _RUNNER


def kernel(x: np.ndarray) -> np.ndarray:
    x = np.asarray(x, dtype=np.float32)
    assert x.shape == (B, Z, A, C), x.shape
    try:
        return _get_runner()(x)
    except Exception:
        # fallback: reference-quality but slower dispatch path
        nc = _build()
        in_maps = [{"x": np.ascontiguousarray(x[i * LB:(i + 1) * LB])}
                   for i in range(N_CORES)]
        res = run_bass_kernel_spmd(nc, in_maps, list(range(N_CORES)))
        return np.concatenate(
            [res.results[i]["out"] for i in range(N_CORES)], axis=0)
